# revision 7
# baseline (speedup 1.0000x reference)
"""Trainium2 Bass kernel for CompositionalFC (moe_routing).

Reference computation:
    z[n,b,o] = x[b,i] @ weight[n,i,o] + bias[n,o]
    out[b,o] = relu( sum_n comp_weight[b,n] * z[n,b,o] )

Strategy: data-parallel over batch across 8 NeuronCores (512 rows each,
weight/bias replicated), with the expert matmuls in fp8e4 DoubleRow mode
(2 contraction rows per PE pass = 2x bf16 matmul throughput, and half the
weight DMA traffic).

Accuracy: fp8e4 has a 3-bit mantissa, too coarse for w ~ U[0,1) directly
(~3.4% rel err vs the 2e-2 gate). Mean-centering fixes it: w = 0.5 + v
with v ~ U[-.5,.5); quantize v to fp8 and add the exact rank-1 term
    0.5 * rowsum(x)[b] * (sum_n c[b,n]),
which also dominates the output magnitude. x ships as fp8 pair
x = xh + xl; the main pass uses xh only, while rowsum(x) is recovered as
rowsum(xh) + rowsum(xl) on device via ones-stationary DoubleRow matmuls
(single LdWeights, output [1, 512] transposed to [128, 4] by a small
SBUF->SBUF DMA). Measured end-to-end l2 rel err: 7.4e-3.

Per core: z_n accumulates in PSUM over 4 DoubleRow K-tiles of 256, then
one fused DVE op per expert: acc = z*c[:,n] + acc. The bias term
(comp_weight @ bias) seeds the accumulators via K=16 bf16 matmuls, the
rank-1 term enters as a per-partition scalar add. ReLU on the way out.

Each stationary xh tile serves 2 experts x 2 PSUM banks (4 matmuls per
LdWeights); the 8 PSUM banks split 4/4 between two (pair, bt) groups so
DVE drain of one overlaps PE streaming of the next. Weight tiles use a
4-deep pool (2 pairs resident) so each pair's DMA hides under the
previous pair's matmuls.
"""

import sys

for _p in ("/opt/trn_rl_repo",):
    if _p not in sys.path:
        sys.path.insert(0, _p)

from contextlib import ExitStack

import ml_dtypes
import numpy as np

import concourse.bass as bass
import concourse.mybir as mybir
import concourse.tile as tile
from concourse import bacc
from concourse.bass_utils import run_bass_kernel_spmd

N_CORES = 8
BATCH, IN_DIM, OUT_DIM, N_EXP = 4096, 1024, 1024, 16
BS = BATCH // N_CORES          # 512 batch rows per core
P = 128                        # partitions
BT = BS // P                   # 4 batch tiles per core
KT2 = IN_DIM // 256            # 4 DoubleRow contraction tiles (K=256 each)
FD = 512                       # matmul free dim / PSUM bank width (fp32)
NO = OUT_DIM // FD             # 2 output column tiles
NPAIR = N_EXP // 2             # expert pairs sharing a stationary tile

F32 = mybir.dt.float32
BF16 = mybir.dt.bfloat16
F8 = mybir.dt.float8e4
DR = mybir.MatmulPerfMode.DoubleRow

E4NP = ml_dtypes.float8_e4m3   # TRN fp8e4 == IEEE e4m3 (max 240)


def _build_kernel():
    nc = bacc.Bacc(
        "TRN2",
        target_bir_lowering=False,
        debug=False,
        num_devices=N_CORES,
    )
    # k = kt2*256 + slot*128 + p; b = bt*128 + p_out
    xh8 = nc.declare_dram_parameter("xh8", [P, KT2, 2, BS], F8, isOutput=False)
    xl8 = nc.declare_dram_parameter("xl8", [P, KT2, 2, BS], F8, isOutput=False)
    w8 = nc.declare_dram_parameter("w8", [N_EXP, P, KT2, 2, OUT_DIM], F8, isOutput=False)
    c = nc.declare_dram_parameter("c", [P, BT, N_EXP], F32, isOutput=False)
    cT = nc.declare_dram_parameter("cT", [N_EXP, BS], BF16, isOutput=False)
    bias = nc.declare_dram_parameter("bias", [N_EXP, OUT_DIM], BF16, isOutput=False)
    out = nc.declare_dram_parameter("out", [P, BT, OUT_DIM], F32, isOutput=True)

    with ExitStack() as ctx:
        tc = ctx.enter_context(tile.TileContext(nc))
        const = ctx.enter_context(tc.tile_pool(name="const", bufs=1))
        accp = ctx.enter_context(tc.tile_pool(name="accp", bufs=1))
        wpool = ctx.enter_context(tc.tile_pool(name="wpool", bufs=4))
        psum = ctx.enter_context(tc.tile_pool(name="psum", bufs=2, space="PSUM"))

        # --- persistent SBUF state -------------------------------------
        # DMA order = startup critical path: tiny seeds, xl+xh (rowsum can
        # finish early), then w[0] chunks gating the first main matmul.
        cT_sb = const.tile([N_EXP, BS], BF16, tag="cT_sb")
        nc.sync.dma_start(cT_sb[:], cT[:, :])
        bias_sb = const.tile([N_EXP, OUT_DIM], BF16, tag="bias_sb")
        nc.sync.dma_start(bias_sb[:], bias[:, :])
        c_sb = const.tile([P, BT, N_EXP], F32, tag="c_sb")
        nc.sync.dma_start(c_sb[:], c[:, :])
        xl_sb = const.tile([P, KT2, 2, BS], F8, tag="xl_sb")
        nc.sync.dma_start(xl_sb[:], xl8[:, :])
        xh_sb = const.tile([P, KT2, 2, BS], F8, tag="xh_sb")
        nc.sync.dma_start(xh_sb[:], xh8[:, :])

        # [P, 2, 16] with col 0 sliced: DoubleRow LdWeights requires the
        # outer free-dim stride to be even and 16B-aligned.
        ones8 = const.tile([P, 2, 16], F8, tag="ones8")
        nc.vector.memset(ones8[:], 1.0)
        sc_sb = const.tile([P, BT], F32, tag="sc_sb")
        nc.vector.tensor_reduce(
            sc_sb[:], c_sb[:], axis=mybir.AxisListType.X, op=mybir.AluOpType.add
        )
        rs_row = const.tile([1, BS], F32, tag="rs_row")
        rs_pb = const.tile([P, BT], F32, tag="rs_pb")
        r1_sb = const.tile([P, BT], F32, tag="r1_sb")

        acc = [
            accp.tile([P, NO, FD], F32, name=f"acc_{bt}", tag=f"acc_{bt}")
            for bt in range(BT)
        ]

        w_sb = {}

        def fetch_pair(pr):
            for e in range(2):
                n = pr * 2 + e
                wt = wpool.tile([P, KT2, 2, OUT_DIM], F8, name=f"w_{n}", tag="w_sb")
                for kt in range(KT2):
                    nc.sync.dma_start(wt[:, kt], w8[n, :, :][:, kt])
                w_sb[n] = wt

        fetch_pair(0)
        fetch_pair(1)

        # --- bias seed: pt = (c @ bias) per bt, K=16 bf16 matmuls -------
        seed_pt = []
        for half in range(2):
            pt = psum.tile([P, 2, NO, FD], F32, name=f"seed_{half}", tag="zp")
            for e in range(2):
                bt = half * 2 + e
                for ot in range(NO):
                    nc.tensor.matmul(
                        pt[:, e, ot],
                        lhsT=cT_sb[:, bt * P : (bt + 1) * P],
                        rhs=bias_sb[:, ot * FD : (ot + 1) * FD],
                        start=True,
                        stop=True,
                    )
            seed_pt.append(pt)
        for bt in range(BT):
            nc.vector.tensor_copy(acc[bt][:], seed_pt[bt // 2][:, bt % 2])

        # --- rowsum(x) = rowsum(xh) + rowsum(xl), ones-stationary -------
        # out lands [1, 512] on partition 0; transpose to [128, 4] via a
        # small SBUF->SBUF DMA. r1 is consumed only at the very end (added
        # before the last combine + relu), keeping it off the startup
        # critical path.
        rs_pt = psum.tile([P, 2, NO, FD], F32, name="rs", tag="zp")
        n_rs = 2 * KT2
        i_rs = 0
        for xt in (xh_sb, xl_sb):
            for kt in range(KT2):
                nc.tensor.matmul(
                    rs_pt[0:1, 0, 0, :],
                    lhsT=ones8[:, :, 0:1],
                    rhs=xt[:, kt],
                    start=(i_rs == 0),
                    stop=(i_rs == n_rs - 1),
                    perf_mode=DR,
                )
                i_rs += 1
        nc.vector.tensor_copy(rs_row[:], rs_pt[0:1, 0, 0, :])
        for bt in range(BT):
            nc.sync.dma_start(
                rs_pb[:, bt : bt + 1], rs_row[0:1, bt * P : (bt + 1) * P]
            )
        # r1 = 0.5 * rowsum * sum_c
        nc.vector.scalar_tensor_tensor(
            out=r1_sb[:],
            in0=rs_pb[:],
            scalar=0.5,
            in1=sc_sb[:],
            op0=mybir.AluOpType.mult,
            op1=mybir.AluOpType.mult,
        )

        # --- main expert-pair loop -------------------------------------
        out_ap = out[:, :]
        for pr in range(NPAIR):
            last_pair = pr == NPAIR - 1
            for bt in range(BT):
                if last_pair:
                    # fold in the rank-1 term before the final combine
                    nc.vector.tensor_scalar(
                        out=acc[bt][:],
                        in0=acc[bt][:],
                        scalar1=r1_sb[:, bt : bt + 1],
                        scalar2=None,
                        op0=mybir.AluOpType.add,
                    )
                zp = psum.tile([P, 2, NO, FD], F32, name="zp", tag="zp")
                for kt in range(KT2):
                    for e in range(2):
                        for ot in range(NO):
                            nc.tensor.matmul(
                                zp[:, e, ot],
                                lhsT=xh_sb[:, kt, :, bt * P : (bt + 1) * P],
                                rhs=w_sb[pr * 2 + e][:, kt, :, ot * FD : (ot + 1) * FD],
                                start=(kt == 0),
                                stop=(kt == KT2 - 1),
                                perf_mode=DR,
                            )
                for e in range(2):
                    n = pr * 2 + e
                    if not (last_pair and e == 1):
                        nc.vector.scalar_tensor_tensor(
                            out=acc[bt][:],
                            in0=zp[:, e],
                            scalar=c_sb[:, bt, n : n + 1],
                            in1=acc[bt][:],
                            op0=mybir.AluOpType.mult,
                            op1=mybir.AluOpType.add,
                        )
                    else:
                        # last expert: fuse combine + relu + store per ot
                        for ot in range(NO):
                            nc.vector.scalar_tensor_tensor(
                                out=acc[bt][:, ot],
                                in0=zp[:, e, ot],
                                scalar=c_sb[:, bt, n : n + 1],
                                in1=acc[bt][:, ot],
                                op0=mybir.AluOpType.mult,
                                op1=mybir.AluOpType.add,
                            )
                            nc.vector.tensor_relu(acc[bt][:, ot], acc[bt][:, ot])
                            nc.sync.dma_start(
                                out_ap[:, bt, ot * FD : (ot + 1) * FD],
                                acc[bt][:, ot],
                            )
            # prefetch two pairs ahead: emitted after this pair's matmuls
            # so the pool-slot WAR dependency sees the right readers.
            if pr + 2 < NPAIR:
                fetch_pair(pr + 2)

    nc.compile()
    return nc


_NC_CACHE = {}


def _get_nc():
    if "nc" not in _NC_CACHE:
        _NC_CACHE["nc"] = _build_kernel()
    return _NC_CACHE["nc"]


def _xt_layout(x8):
    # fp8 [BS, IN_DIM] -> lhsT [P, KT2, 2, BS] with k = kt2*256+slot*128+p
    xT = np.ascontiguousarray(x8.T)  # [IN_DIM, BS]
    return np.ascontiguousarray(xT.reshape(KT2, 2, P, BS).transpose(2, 0, 1, 3))


def prepare_inputs(x, comp_weight, weight, bias):
    x = np.ascontiguousarray(np.asarray(x, dtype=np.float32))
    comp_weight = np.ascontiguousarray(np.asarray(comp_weight, dtype=np.float32))
    weight = np.asarray(weight, dtype=np.float32)
    bias = np.ascontiguousarray(np.asarray(bias, dtype=np.float32))

    # w = 0.5 + v; ship v in fp8 laid out [n, p, kt2, slot, o]
    v8 = (weight - np.float32(0.5)).astype(E4NP)
    w8 = np.ascontiguousarray(
        v8.reshape(N_EXP, KT2, 2, P, OUT_DIM).transpose(0, 3, 1, 2, 4)
    )
    bias_bf = bias.astype(ml_dtypes.bfloat16)

    in_maps = []
    for r in range(N_CORES):
        sl = slice(r * BS, (r + 1) * BS)
        xs = x[sl]
        cs = comp_weight[sl]
        xh = xs.astype(E4NP)
        xl = (xs - xh.astype(np.float32)).astype(E4NP)
        in_maps.append(
            {
                "xh8": _xt_layout(xh),
                "xl8": _xt_layout(xl),
                "w8": w8,
                "c": np.ascontiguousarray(cs.reshape(BT, P, N_EXP).transpose(1, 0, 2)),
                "cT": np.ascontiguousarray(cs.T).astype(ml_dtypes.bfloat16),
                "bias": bias_bf,
            }
        )
    return in_maps


def _run(x, comp_weight, weight, bias, trace=False):
    in_maps = prepare_inputs(x, comp_weight, weight, bias)
    res = run_bass_kernel_spmd(
        _get_nc(), in_maps, core_ids=list(range(N_CORES)), trace=trace
    )
    out = np.concatenate(
        [
            res.results[r]["out"].transpose(1, 0, 2).reshape(BS, OUT_DIM)
            for r in range(N_CORES)
        ],
        axis=0,
    )
    return out, res


def kernel(x, comp_weight, weight, bias):
    out, _ = _run(x, comp_weight, weight, bias)
    return out


# revision 9
# speedup vs baseline: 1.0232x; 1.0232x over previous
"""Trainium2 Bass kernel for CompositionalFC (moe_routing).

Reference computation:
    z[n,b,o] = x[b,i] @ weight[n,i,o] + bias[n,o]
    out[b,o] = relu( sum_n comp_weight[b,n] * z[n,b,o] )

Strategy: data-parallel over batch across 8 NeuronCores (512 rows each,
weight/bias replicated), with the expert matmuls in fp8e4 DoubleRow mode
(2 contraction rows per PE pass = 2x bf16 matmul throughput, and half the
weight DMA traffic). Steady state measured at 216 ns per 512-col DoubleRow
matmul == the fp8 PE roofline (~157 TF/s effective).

Accuracy: fp8e4 has a 3-bit mantissa, too coarse for w ~ U[0,1) directly
(~3.4% rel err vs the 2e-2 gate). Mean-centering fixes it: w = 0.5 + v
with v ~ U[-.5,.5); quantize v to fp8 and add the exact rank-1 term
    0.5 * rowsum(x)[b] * (sum_n c[b,n]),
which also dominates the output magnitude. x ships as fp8 pair
x = xh + xl; the main pass uses xh only, while rowsum(x) is recovered as
rowsum(xh) + rowsum(xl) on device via ones-stationary DoubleRow matmuls
(single LdWeights, output [1, 512] transposed to [128, 4] by small
SBUF->SBUF DMAs). Measured end-to-end l2 rel err: 7.3e-3.

Per core: z_n accumulates in PSUM over 4 DoubleRow K-tiles of 256, then
one fused combine op per expert: acc = z*c[:,n] + acc. The bias term
(comp_weight @ bias) seeds the accumulators via K=16 bf16 matmuls (hidden
under the startup DMA window); the rank-1 term is added after pair 1,
off both the startup and drain critical paths. ReLU on the way out.

Engine placement: combines (PSUM readers) live on the Vector engine;
the Scalar engine seeds the accumulators from PSUM and fuses the rank-1
term into the final ReLU (bias AP), keeping the drain chain short.
GpSimd cannot access PSUM on TRN2. Each stationary xh tile serves
2 experts x 2 PSUM banks (4 matmuls per LdWeights); the 8 PSUM banks
split 4/4 between two (pair, bt) groups so combine drain overlaps PE
streaming. Weight pair DMAs interleave the two experts' K-chunks (the
kt loop needs both experts' chunk kt first), and pairs 2+ prefetch two
pairs ahead through a 4-deep pool.
"""

import sys

for _p in ("/opt/trn_rl_repo",):
    if _p not in sys.path:
        sys.path.insert(0, _p)

from contextlib import ExitStack

import ml_dtypes
import numpy as np

import concourse.bass as bass
import concourse.mybir as mybir
import concourse.tile as tile
from concourse import bacc
from concourse.bass_utils import run_bass_kernel_spmd

N_CORES = 8
BATCH, IN_DIM, OUT_DIM, N_EXP = 4096, 1024, 1024, 16
BS = BATCH // N_CORES          # 512 batch rows per core
P = 128                        # partitions
BT = BS // P                   # 4 batch tiles per core
KT2 = IN_DIM // 256            # 4 DoubleRow contraction tiles (K=256 each)
FD = 512                       # matmul free dim / PSUM bank width (fp32)
NO = OUT_DIM // FD             # 2 output column tiles
NPAIR = N_EXP // 2             # expert pairs sharing a stationary tile

F32 = mybir.dt.float32
BF16 = mybir.dt.bfloat16
F8 = mybir.dt.float8e4
DR = mybir.MatmulPerfMode.DoubleRow

E4NP = ml_dtypes.float8_e4m3   # TRN fp8e4 == IEEE e4m3 (max 240)


def _build_kernel():
    nc = bacc.Bacc(
        "TRN2",
        target_bir_lowering=False,
        debug=False,
        num_devices=N_CORES,
    )
    # k = kt2*256 + slot*128 + p; b = bt*128 + p_out
    xh8 = nc.declare_dram_parameter("xh8", [P, KT2, 2, BS], F8, isOutput=False)
    xl8 = nc.declare_dram_parameter("xl8", [P, KT2, 2, BS], F8, isOutput=False)
    w8 = nc.declare_dram_parameter("w8", [N_EXP, P, KT2, 2, OUT_DIM], F8, isOutput=False)
    c = nc.declare_dram_parameter("c", [P, BT, N_EXP], F32, isOutput=False)
    cT = nc.declare_dram_parameter("cT", [N_EXP, BS], BF16, isOutput=False)
    bias = nc.declare_dram_parameter("bias", [N_EXP, OUT_DIM], BF16, isOutput=False)
    out = nc.declare_dram_parameter("out", [P, BT, OUT_DIM], F32, isOutput=True)

    with ExitStack() as ctx:
        tc = ctx.enter_context(tile.TileContext(nc))
        const = ctx.enter_context(tc.tile_pool(name="const", bufs=1))
        accp = ctx.enter_context(tc.tile_pool(name="accp", bufs=1))
        wpool = ctx.enter_context(tc.tile_pool(name="wpool", bufs=4))
        psum = ctx.enter_context(tc.tile_pool(name="psum", bufs=2, space="PSUM"))

        # --- persistent SBUF state -------------------------------------
        # DMA issue order IS the startup critical path (each dma_start
        # costs ~650ns of sync-queue issue time): cT+bias gate the seeds,
        # xh + the first interleaved w chunks gate the main loop; c/xl are
        # needed much later (rowsum runs after pair 0).
        cT_sb = const.tile([N_EXP, BS], BF16, tag="cT_sb")
        nc.sync.dma_start(cT_sb[:], cT[:, :])
        bias_sb = const.tile([N_EXP, OUT_DIM], BF16, tag="bias_sb")
        nc.sync.dma_start(bias_sb[:], bias[:, :])
        xh_sb = const.tile([P, KT2, 2, BS], F8, tag="xh_sb")
        nc.sync.dma_start(xh_sb[:], xh8[:, :])

        ones8 = const.tile([P, 2, 16], F8, tag="ones8")
        nc.vector.memset(ones8[:], 1.0)
        rs_row = const.tile([1, BS], F32, tag="rs_row")
        rs_pb = const.tile([P, BT], F32, tag="rs_pb")
        r1_sb = const.tile([P, BT], F32, tag="r1_sb")
        sc_sb = const.tile([P, BT], F32, tag="sc_sb")

        acc = [
            accp.tile([P, NO, FD], F32, name=f"acc_{bt}", tag=f"acc_{bt}")
            for bt in range(BT)
        ]

        w_sb = {}

        def fetch_pair(pr, chunked):
            n0, n1 = pr * 2, pr * 2 + 1
            for n in (n0, n1):
                w_sb[n] = wpool.tile(
                    [P, KT2, 2, OUT_DIM], F8, name=f"w_{n}", tag="w_sb"
                )
            if chunked:
                # interleave the experts' K-chunks: the kt loop needs both
                # experts' chunk kt before it can proceed
                for kt in range(KT2):
                    for n in (n0, n1):
                        nc.sync.dma_start(w_sb[n][:, kt], w8[n, :, :][:, kt])
            else:
                for n in (n0, n1):
                    nc.sync.dma_start(w_sb[n][:], w8[n, :, :])

        fetch_pair(0, chunked=True)

        c_sb = const.tile([P, BT, N_EXP], F32, tag="c_sb")
        nc.sync.dma_start(c_sb[:], c[:, :])
        xl_sb = const.tile([P, KT2, 2, BS], F8, tag="xl_sb")
        nc.sync.dma_start(xl_sb[:], xl8[:, :])

        fetch_pair(1, chunked=True)

        nc.vector.tensor_reduce(
            sc_sb[:], c_sb[:], axis=mybir.AxisListType.X, op=mybir.AluOpType.add
        )

        # --- bias seed: pt = (c @ bias) per bt, K=16 bf16 matmuls -------
        # Runs in the startup DMA window while xh/w0 stream in.
        seed_pt = []
        for half in range(2):
            pt = psum.tile([P, 2, NO, FD], F32, name=f"seed_{half}", tag="zp")
            for e in range(2):
                bt = half * 2 + e
                for ot in range(NO):
                    nc.tensor.matmul(
                        pt[:, e, ot],
                        lhsT=cT_sb[:, bt * P : (bt + 1) * P],
                        rhs=bias_sb[:, ot * FD : (ot + 1) * FD],
                        start=True,
                        stop=True,
                    )
            seed_pt.append(pt)
        for bt in range(BT):
            nc.scalar.activation(
                acc[bt][:],
                seed_pt[bt // 2][:, bt % 2],
                mybir.ActivationFunctionType.Copy,
            )

        # --- main expert-pair loop -------------------------------------
        out_ap = out[:, :]
        for pr in range(NPAIR):
            last_pair = pr == NPAIR - 1
            for bt in range(BT):
                zp = psum.tile([P, 2, NO, FD], F32, name="zp", tag="zp")
                for kt in range(KT2):
                    for e in range(2):
                        for ot in range(NO):
                            nc.tensor.matmul(
                                zp[:, e, ot],
                                lhsT=xh_sb[:, kt, :, bt * P : (bt + 1) * P],
                                rhs=w_sb[pr * 2 + e][:, kt, :, ot * FD : (ot + 1) * FD],
                                start=(kt == 0),
                                stop=(kt == KT2 - 1),
                                perf_mode=DR,
                            )
                for e in range(2):
                    n = pr * 2 + e
                    if not (last_pair and e == 1):
                        nc.vector.scalar_tensor_tensor(
                            out=acc[bt][:],
                            in0=zp[:, e],
                            scalar=c_sb[:, bt, n : n + 1],
                            in1=acc[bt][:],
                            op0=mybir.AluOpType.mult,
                            op1=mybir.AluOpType.add,
                        )
                    else:
                        # last expert: combine + relu + store per ot
                        for ot in range(NO):
                            nc.vector.scalar_tensor_tensor(
                                out=acc[bt][:, ot],
                                in0=zp[:, e, ot],
                                scalar=c_sb[:, bt, n : n + 1],
                                in1=acc[bt][:, ot],
                                op0=mybir.AluOpType.mult,
                                op1=mybir.AluOpType.add,
                            )
                            nc.scalar.activation(
                                acc[bt][:, ot],
                                acc[bt][:, ot],
                                mybir.ActivationFunctionType.Relu,
                                bias=r1_sb[:, bt : bt + 1],
                            )
                            nc.sync.dma_start(
                                out_ap[:, bt, ot * FD : (ot + 1) * FD],
                                acc[bt][:, ot],
                            )

            if pr == 0:
                # --- rowsum(x) = rowsum(xh) + rowsum(xl) ---------------
                # ones-stationary DoubleRow matmuls -> [1, 512] on
                # partition 0, transposed to [128, 4] via small DMAs.
                # Emitted after pair 0 so it never gates the startup; r1
                # is consumed only by the final ReLU bias.
                rs_pt = psum.tile([P, 2, NO, FD], F32, name="rs", tag="zp")
                n_rs = 2 * KT2
                i_rs = 0
                for xt in (xh_sb, xl_sb):
                    for kt in range(KT2):
                        nc.tensor.matmul(
                            rs_pt[0:1, 0, 0, :],
                            lhsT=ones8[:, :, 0:1],
                            rhs=xt[:, kt],
                            start=(i_rs == 0),
                            stop=(i_rs == n_rs - 1),
                            perf_mode=DR,
                        )
                        i_rs += 1
                nc.vector.tensor_copy(rs_row[:], rs_pt[0:1, 0, 0, :])
                for bt in range(BT):
                    nc.sync.dma_start(
                        rs_pb[:, bt : bt + 1], rs_row[0:1, bt * P : (bt + 1) * P]
                    )
                # r1 = 0.5 * rowsum * sum_c
                nc.vector.scalar_tensor_tensor(
                    out=r1_sb[:],
                    in0=rs_pb[:],
                    scalar=0.5,
                    in1=sc_sb[:],
                    op0=mybir.AluOpType.mult,
                    op1=mybir.AluOpType.mult,
                )

            # prefetch two pairs ahead: emitted after this pair's matmuls
            # so the pool-slot WAR dependency sees the right readers.
            if pr + 2 < NPAIR:
                fetch_pair(pr + 2, chunked=False)

    nc.compile()
    return nc


_NC_CACHE = {}


def _get_nc():
    if "nc" not in _NC_CACHE:
        _NC_CACHE["nc"] = _build_kernel()
    return _NC_CACHE["nc"]


def _xt_layout(x8):
    # fp8 [BS, IN_DIM] -> lhsT [P, KT2, 2, BS] with k = kt2*256+slot*128+p
    xT = np.ascontiguousarray(x8.T)  # [IN_DIM, BS]
    return np.ascontiguousarray(xT.reshape(KT2, 2, P, BS).transpose(2, 0, 1, 3))


def prepare_inputs(x, comp_weight, weight, bias):
    x = np.ascontiguousarray(np.asarray(x, dtype=np.float32))
    comp_weight = np.ascontiguousarray(np.asarray(comp_weight, dtype=np.float32))
    weight = np.asarray(weight, dtype=np.float32)
    bias = np.ascontiguousarray(np.asarray(bias, dtype=np.float32))

    # w = 0.5 + v; ship v in fp8 laid out [n, p, kt2, slot, o]
    v8 = (weight - np.float32(0.5)).astype(E4NP)
    w8 = np.ascontiguousarray(
        v8.reshape(N_EXP, KT2, 2, P, OUT_DIM).transpose(0, 3, 1, 2, 4)
    )
    bias_bf = bias.astype(ml_dtypes.bfloat16)

    in_maps = []
    for r in range(N_CORES):
        sl = slice(r * BS, (r + 1) * BS)
        xs = x[sl]
        cs = comp_weight[sl]
        xh = xs.astype(E4NP)
        xl = (xs - xh.astype(np.float32)).astype(E4NP)
        in_maps.append(
            {
                "xh8": _xt_layout(xh),
                "xl8": _xt_layout(xl),
                "w8": w8,
                "c": np.ascontiguousarray(cs.reshape(BT, P, N_EXP).transpose(1, 0, 2)),
                "cT": np.ascontiguousarray(cs.T).astype(ml_dtypes.bfloat16),
                "bias": bias_bf,
            }
        )
    return in_maps


def _run(x, comp_weight, weight, bias, trace=False):
    in_maps = prepare_inputs(x, comp_weight, weight, bias)
    res = run_bass_kernel_spmd(
        _get_nc(), in_maps, core_ids=list(range(N_CORES)), trace=trace
    )
    out = np.concatenate(
        [
            res.results[r]["out"].transpose(1, 0, 2).reshape(BS, OUT_DIM)
            for r in range(N_CORES)
        ],
        axis=0,
    )
    return out, res


def kernel(x, comp_weight, weight, bias):
    out, _ = _run(x, comp_weight, weight, bias)
    return out


# revision 10
# speedup vs baseline: 1.0411x; 1.0175x over previous
"""Trainium2 Bass kernel for CompositionalFC (moe_routing).

Reference computation:
    z[n,b,o] = x[b,i] @ weight[n,i,o] + bias[n,o]
    out[b,o] = relu( sum_n comp_weight[b,n] * z[n,b,o] )

Strategy: data-parallel over batch across 8 NeuronCores (512 rows each,
weight/bias replicated), with the expert matmuls in fp8e4 DoubleRow mode
(2 contraction rows per PE pass = 2x bf16 matmul throughput, and half the
weight DMA traffic). Steady state measured at 216 ns per 512-col DoubleRow
matmul == the fp8 PE roofline (~157 TF/s effective).

Accuracy: fp8e4 has a 3-bit mantissa, too coarse for w ~ U[0,1) directly
(~3.4% rel err vs the 2e-2 gate). Mean-centering fixes it: w = 0.5 + v
with v ~ U[-.5,.5); quantize v to fp8 and add the exact rank-1 term
    0.5 * rowsum(x)[b] * (sum_n c[b,n]),
which also dominates the output magnitude. x ships as fp8 pair
x = xh + xl; the main pass uses xh only, while rowsum(x) is recovered as
rowsum(xh) + rowsum(xl) on device via ones-stationary DoubleRow matmuls
(single LdWeights, output [1, 512] transposed to [128, 4] by small
SBUF->SBUF DMAs). Measured end-to-end l2 rel err: 7.3e-3.

Per core: z_n accumulates in PSUM over 4 DoubleRow K-tiles of 256, then
one fused combine op per expert: acc = z*c[:,n] + acc. The bias term
(comp_weight @ bias) seeds the accumulators via K=16 bf16 matmuls (hidden
under the startup DMA window); the rank-1 term is added after pair 1,
off both the startup and drain critical paths. ReLU on the way out.

Engine placement: combines (PSUM readers) live on the Vector engine;
the Scalar engine seeds the accumulators from PSUM and fuses the rank-1
term into the final ReLU (bias AP), keeping the drain chain short.
GpSimd cannot access PSUM on TRN2. Each stationary xh tile serves
2 experts x 2 PSUM banks (4 matmuls per LdWeights); the 8 PSUM banks
split 4/4 between two (pair, bt) groups so combine drain overlaps PE
streaming. Weight pair DMAs interleave the two experts' K-chunks (the
kt loop needs both experts' chunk kt first), and pairs 2+ prefetch two
pairs ahead through a 4-deep pool.
"""

import sys

for _p in ("/opt/trn_rl_repo",):
    if _p not in sys.path:
        sys.path.insert(0, _p)

from contextlib import ExitStack

import ml_dtypes
import numpy as np

import concourse.bass as bass
import concourse.mybir as mybir
import concourse.tile as tile
from concourse import bacc
from concourse.bass_utils import run_bass_kernel_spmd

N_CORES = 8
BATCH, IN_DIM, OUT_DIM, N_EXP = 4096, 1024, 1024, 16
BS = BATCH // N_CORES          # 512 batch rows per core
P = 128                        # partitions
BT = BS // P                   # 4 batch tiles per core
KT2 = IN_DIM // 256            # 4 DoubleRow contraction tiles (K=256 each)
FD = 512                       # matmul free dim / PSUM bank width (fp32)
NO = OUT_DIM // FD             # 2 output column tiles
NPAIR = N_EXP // 2             # expert pairs sharing a stationary tile

F32 = mybir.dt.float32
BF16 = mybir.dt.bfloat16
F8 = mybir.dt.float8e4
DR = mybir.MatmulPerfMode.DoubleRow

E4NP = ml_dtypes.float8_e4m3   # TRN fp8e4 == IEEE e4m3 (max 240)


def _build_kernel():
    nc = bacc.Bacc(
        "TRN2",
        target_bir_lowering=False,
        debug=False,
        num_devices=N_CORES,
    )
    # k = kt2*256 + slot*128 + p; b = bt*128 + p_out
    xh8 = nc.declare_dram_parameter("xh8", [P, KT2, 2, BS], F8, isOutput=False)
    xl8 = nc.declare_dram_parameter("xl8", [P, KT2, 2, BS], F8, isOutput=False)
    w8 = nc.declare_dram_parameter("w8", [N_EXP, P, KT2, 2, OUT_DIM], F8, isOutput=False)
    c = nc.declare_dram_parameter("c", [P, BT, N_EXP], F32, isOutput=False)
    cT = nc.declare_dram_parameter("cT", [N_EXP, BS], BF16, isOutput=False)
    bias = nc.declare_dram_parameter("bias", [N_EXP, OUT_DIM], BF16, isOutput=False)
    out = nc.declare_dram_parameter("out", [P, BT, OUT_DIM], F32, isOutput=True)

    with ExitStack() as ctx:
        tc = ctx.enter_context(tile.TileContext(nc))
        const = ctx.enter_context(tc.tile_pool(name="const", bufs=1))
        accp = ctx.enter_context(tc.tile_pool(name="accp", bufs=1))
        wpool = ctx.enter_context(tc.tile_pool(name="wpool", bufs=4))
        psum = ctx.enter_context(tc.tile_pool(name="psum", bufs=2, space="PSUM"))

        # --- persistent SBUF state -------------------------------------
        # DMA issue order IS the startup critical path (each dma_start
        # costs ~650ns of sync-queue issue time): cT+bias gate the seeds,
        # xh + the first interleaved w chunks gate the main loop; c/xl are
        # needed much later (rowsum runs after pair 0).
        # seeds' operands issue on the (otherwise idle) GpSimd queue so
        # they land in parallel with xh/w on the sync queue
        cT_sb = const.tile([N_EXP, BS], BF16, tag="cT_sb")
        nc.gpsimd.dma_start(cT_sb[:], cT[:, :])
        bias_sb = const.tile([N_EXP, OUT_DIM], BF16, tag="bias_sb")
        nc.gpsimd.dma_start(bias_sb[:], bias[:, :])
        xh_sb = const.tile([P, KT2, 2, BS], F8, tag="xh_sb")
        nc.sync.dma_start(xh_sb[:], xh8[:, :])

        ones8 = const.tile([P, 2, 16], F8, tag="ones8")
        nc.vector.memset(ones8[:], 1.0)
        junk8 = const.tile([P, 2, FD], F8, tag="junk8")
        nc.vector.memset(junk8[:], 1.0)
        rs_row = const.tile([1, BS], F32, tag="rs_row")
        rs_pb = const.tile([P, BT], F32, tag="rs_pb")
        r1_sb = const.tile([P, BT], F32, tag="r1_sb")
        sc_sb = const.tile([P, BT], F32, tag="sc_sb")

        acc = [
            accp.tile([P, NO, FD], F32, name=f"acc_{bt}", tag=f"acc_{bt}")
            for bt in range(BT)
        ]

        w_sb = {}

        def fetch_pair(pr, chunked):
            n0, n1 = pr * 2, pr * 2 + 1
            for n in (n0, n1):
                w_sb[n] = wpool.tile(
                    [P, KT2, 2, OUT_DIM], F8, name=f"w_{n}", tag="w_sb"
                )
            if chunked:
                # interleave the experts' K-chunks: the kt loop needs both
                # experts' chunk kt before it can proceed
                for kt in range(KT2):
                    for n in (n0, n1):
                        nc.sync.dma_start(w_sb[n][:, kt], w8[n, :, :][:, kt])
            else:
                for n in (n0, n1):
                    nc.sync.dma_start(w_sb[n][:], w8[n, :, :])

        fetch_pair(0, chunked=True)

        c_sb = const.tile([P, BT, N_EXP], F32, tag="c_sb")
        nc.sync.dma_start(c_sb[:], c[:, :])
        xl_sb = const.tile([P, KT2, 2, BS], F8, tag="xl_sb")
        nc.sync.dma_start(xl_sb[:], xl8[:, :])

        fetch_pair(1, chunked=True)

        nc.vector.tensor_reduce(
            sc_sb[:], c_sb[:], axis=mybir.AxisListType.X, op=mybir.AluOpType.add
        )

        # --- PE clock warm-up: keep the PE busy through the DMA window
        # so the seeds and first main matmuls run at full p-state.
        jk = psum.tile([P, 2, NO, FD], F32, name="junk", tag="zp")
        for _ in range(10):
            nc.tensor.matmul(
                jk[0:1, 0, 0, :],
                lhsT=ones8[:, :, 0:1],
                rhs=junk8[:],
                start=True,
                stop=True,
                perf_mode=DR,
            )

        # --- bias seed: pt = (c @ bias) per bt, K=16 bf16 matmuls -------
        # Runs in the startup DMA window while xh/w0 stream in.
        seed_pt = []
        for half in range(2):
            pt = psum.tile([P, 2, NO, FD], F32, name=f"seed_{half}", tag="zp")
            for e in range(2):
                bt = half * 2 + e
                for ot in range(NO):
                    nc.tensor.matmul(
                        pt[:, e, ot],
                        lhsT=cT_sb[:, bt * P : (bt + 1) * P],
                        rhs=bias_sb[:, ot * FD : (ot + 1) * FD],
                        start=True,
                        stop=True,
                    )
            seed_pt.append(pt)
        for bt in range(BT):
            nc.scalar.activation(
                acc[bt][:],
                seed_pt[bt // 2][:, bt % 2],
                mybir.ActivationFunctionType.Copy,
            )

        # --- main expert loop: pairs for 0-13, solo for 14/15 ----------
        # The two solo phases at the end spread the final combine+relu+
        # store chains over the last two expert windows instead of piling
        # all four behind the very last matmuls (which starved PSUM slots
        # and stalled the PE).
        groups = [(2 * p, 2 * p + 1) for p in range(NPAIR - 1)] + [(14,), (15,)]
        out_ap = out[:, :]
        for gi, grp in enumerate(groups):
            for bt in range(BT):
                ne = len(grp)
                zp = psum.tile([P, ne, NO, FD], F32, name="zp", tag="zp")
                for kt in range(KT2):
                    for ei, n in enumerate(grp):
                        for ot in range(NO):
                            nc.tensor.matmul(
                                zp[:, ei, ot],
                                lhsT=xh_sb[:, kt, :, bt * P : (bt + 1) * P],
                                rhs=w_sb[n][:, kt, :, ot * FD : (ot + 1) * FD],
                                start=(kt == 0),
                                stop=(kt == KT2 - 1),
                                perf_mode=DR,
                            )
                for ei, n in enumerate(grp):
                    if n != N_EXP - 1:
                        nc.vector.scalar_tensor_tensor(
                            out=acc[bt][:],
                            in0=zp[:, ei],
                            scalar=c_sb[:, bt, n : n + 1],
                            in1=acc[bt][:],
                            op0=mybir.AluOpType.mult,
                            op1=mybir.AluOpType.add,
                        )
                    else:
                        # last expert: combine + relu(+rank-1) + store per ot
                        for ot in range(NO):
                            nc.vector.scalar_tensor_tensor(
                                out=acc[bt][:, ot],
                                in0=zp[:, ei, ot],
                                scalar=c_sb[:, bt, n : n + 1],
                                in1=acc[bt][:, ot],
                                op0=mybir.AluOpType.mult,
                                op1=mybir.AluOpType.add,
                            )
                            nc.scalar.activation(
                                acc[bt][:, ot],
                                acc[bt][:, ot],
                                mybir.ActivationFunctionType.Relu,
                                bias=r1_sb[:, bt : bt + 1],
                            )
                            nc.sync.dma_start(
                                out_ap[:, bt, ot * FD : (ot + 1) * FD],
                                acc[bt][:, ot],
                            )

            if gi == 0:
                # --- rowsum(x) = rowsum(xh) + rowsum(xl) ---------------
                # ones-stationary DoubleRow matmuls -> [1, 512] on
                # partition 0, transposed to [128, 4] via small DMAs.
                # Emitted after group 0 so it never gates the startup; r1
                # is consumed only by the final ReLU bias.
                rs_pt = psum.tile([P, 2, NO, FD], F32, name="rs", tag="zp")
                n_rs = 2 * KT2
                i_rs = 0
                for xt in (xh_sb, xl_sb):
                    for kt in range(KT2):
                        nc.tensor.matmul(
                            rs_pt[0:1, 0, 0, :],
                            lhsT=ones8[:, :, 0:1],
                            rhs=xt[:, kt],
                            start=(i_rs == 0),
                            stop=(i_rs == n_rs - 1),
                            perf_mode=DR,
                        )
                        i_rs += 1
                nc.vector.tensor_copy(rs_row[:], rs_pt[0:1, 0, 0, :])
                for bt in range(BT):
                    nc.sync.dma_start(
                        rs_pb[:, bt : bt + 1], rs_row[0:1, bt * P : (bt + 1) * P]
                    )
                # r1 = 0.5 * rowsum * sum_c
                nc.vector.scalar_tensor_tensor(
                    out=r1_sb[:],
                    in0=rs_pb[:],
                    scalar=0.5,
                    in1=sc_sb[:],
                    op0=mybir.AluOpType.mult,
                    op1=mybir.AluOpType.mult,
                )

            # prefetch two groups ahead: emitted after this group's
            # matmuls so the pool-slot WAR dependency sees the readers.
            if gi + 2 < len(groups):
                for n in groups[gi + 2]:
                    wt = wpool.tile(
                        [P, KT2, 2, OUT_DIM], F8, name=f"w_{n}", tag="w_sb"
                    )
                    nc.sync.dma_start(wt[:], w8[n, :, :])
                    w_sb[n] = wt

    nc.compile()
    return nc


_NC_CACHE = {}


def _get_nc():
    if "nc" not in _NC_CACHE:
        _NC_CACHE["nc"] = _build_kernel()
    return _NC_CACHE["nc"]


def _xt_layout(x8):
    # fp8 [BS, IN_DIM] -> lhsT [P, KT2, 2, BS] with k = kt2*256+slot*128+p
    xT = np.ascontiguousarray(x8.T)  # [IN_DIM, BS]
    return np.ascontiguousarray(xT.reshape(KT2, 2, P, BS).transpose(2, 0, 1, 3))


def prepare_inputs(x, comp_weight, weight, bias):
    x = np.ascontiguousarray(np.asarray(x, dtype=np.float32))
    comp_weight = np.ascontiguousarray(np.asarray(comp_weight, dtype=np.float32))
    weight = np.asarray(weight, dtype=np.float32)
    bias = np.ascontiguousarray(np.asarray(bias, dtype=np.float32))

    # w = 0.5 + v; ship v in fp8 laid out [n, p, kt2, slot, o]
    v8 = (weight - np.float32(0.5)).astype(E4NP)
    w8 = np.ascontiguousarray(
        v8.reshape(N_EXP, KT2, 2, P, OUT_DIM).transpose(0, 3, 1, 2, 4)
    )
    bias_bf = bias.astype(ml_dtypes.bfloat16)

    in_maps = []
    for r in range(N_CORES):
        sl = slice(r * BS, (r + 1) * BS)
        xs = x[sl]
        cs = comp_weight[sl]
        xh = xs.astype(E4NP)
        xl = (xs - xh.astype(np.float32)).astype(E4NP)
        in_maps.append(
            {
                "xh8": _xt_layout(xh),
                "xl8": _xt_layout(xl),
                "w8": w8,
                "c": np.ascontiguousarray(cs.reshape(BT, P, N_EXP).transpose(1, 0, 2)),
                "cT": np.ascontiguousarray(cs.T).astype(ml_dtypes.bfloat16),
                "bias": bias_bf,
            }
        )
    return in_maps


def _run(x, comp_weight, weight, bias, trace=False):
    in_maps = prepare_inputs(x, comp_weight, weight, bias)
    res = run_bass_kernel_spmd(
        _get_nc(), in_maps, core_ids=list(range(N_CORES)), trace=trace
    )
    out = np.concatenate(
        [
            res.results[r]["out"].transpose(1, 0, 2).reshape(BS, OUT_DIM)
            for r in range(N_CORES)
        ],
        axis=0,
    )
    return out, res


def kernel(x, comp_weight, weight, bias):
    out, _ = _run(x, comp_weight, weight, bias)
    return out
